# revision 25
# baseline (speedup 1.0000x reference)
"""Trainium2 Bass kernel for nn_MixtureOfExperts (moe_routing).

Strategy (expert-parallel + H-split, derived from the sharding hint):
  - Host computes the tiny router (N x D @ D x E = 0.1% of total FLOPs),
    top-k selection and softmax combine weights in fp32 numpy.
  - Experts are paired so their token-tile counts balance (a 9-tile
    expert with an 8-tile one).  Each pair is served by TWO cores, each
    computing one H-half (1024 cols) of BOTH experts of the pair:
        out_e = (x_e @ W_e) * silu(x_e @ V_e)
    This equalizes per-core matmul counts across all 8 cores at
    (T1+T2) token-tiles x (H/2) columns, below the max-expert padding
    cost of plain expert-parallel.  Matmuls run in bf16 (fp32 PSUM).
  - Host scatter-adds the per-expert outputs weighted by the combine
    probabilities (the reference's zeros+scatter-add semantics).

Device kernel layout per core (pair (a,b), h-half h):
  xta  [D, T1*128] bf16   tokens routed to expert a (transposed)
  xtb  [D, T2*128] bf16   tokens routed to expert b
  w,v  [2, 2, D, 512] bf16  h-slab-major weights: [slot, sub-slab, D, 512]
  out  [(T1+T2)*128, 1024] fp32
  Loop: per (expert-slot, 512-col sub-slab), per 128-token tile:
  16 accumulating matmuls per matrix (lhsT = stationary x^T tile,
  rhs = weight slab slice), then ACT silu + DVE multiply, DMA out.
"""

import numpy as np
import ml_dtypes

P = 128
D = 2048
H = 2048
E = 8
N_CORES = 8
HSLAB = 512
HHALF = 1024

_compiled = {}


def _build(T1, T2):
    """Build the Bass/Tile kernel for token-tile counts (T1, T2)."""
    import concourse.bacc as bacc
    import concourse.mybir as mybir
    import concourse.tile as tile

    KT = D // P              # 16 contraction tiles
    C1, C2 = T1 * P, T2 * P
    C = C1 + C2

    nc = bacc.Bacc("TRN2", target_bir_lowering=False, debug=False)
    bf16 = mybir.dt.bfloat16
    f32 = mybir.dt.float32

    # SBUF budget guard (KB/partition): resident x^T costs 4*(T1+T2),
    # each weight-slab buffer 16, elementwise pools ~24.  Shrink slab
    # prefetch depth if a skewed routing inflates the token capacity.
    wv_bufs = 5
    while 4 * (T1 + T2) + 16 * wv_bufs + 28 > 188 and wv_bufs > 2:
        wv_bufs -= 1

    xta = nc.dram_tensor("xta", [D, C1], bf16, kind="ExternalInput").ap()
    xtb = nc.dram_tensor("xtb", [D, C2], bf16, kind="ExternalInput").ap()
    Wd = nc.dram_tensor("w", [2, 2, D, HSLAB], bf16, kind="ExternalInput").ap()
    Vd = nc.dram_tensor("v", [2, 2, D, HSLAB], bf16, kind="ExternalInput").ap()
    # First W/V slab pair pre-fused host-side: doubles bytes-per-dma_start
    # during the ramp chase, which is issue-rate-bound (~0.6us/issue).
    WV0 = nc.dram_tensor("wv0", [D, 2 * HSLAB], bf16, kind="ExternalInput").ap()
    out = nc.dram_tensor("out", [C, HHALF], f32, kind="ExternalOutput").ap()

    with tile.TileContext(nc) as tc:
        with (
            tc.tile_pool(name="xt", bufs=1) as xt_pool,
            tc.tile_pool(name="wv", bufs=wv_bufs) as wv_pool,
            tc.tile_pool(name="elt", bufs=5) as elt_pool,
            tc.tile_pool(name="psum", bufs=8, space="PSUM") as psum_pool,
        ):
            # Resident x^T for both experts of the pair.  Interleave the
            # ramp-critical DMAs (xta, first W slab, first V slab) in
            # d-order so the PE starts the first accumulation chain as
            # data arrives; xtb streams later (needed after T1 tiles).
            xta_sb = xt_pool.tile([P, KT, C1], bf16, tag="xta")
            xtb_sb = xt_pool.tile([P, KT, C2], bf16, tag="xtb")
            wv0_sb = wv_pool.tile([P, KT, 2 * HSLAB], bf16, tag="wv0", bufs=1)

            # HAM pre-warm v2: back-to-back N=128 matmuls ALTERNATING psum
            # banks (a single bank serializes on WAW+drain and never trips
            # the activity window).  ~3.4us of dense PE activity during the
            # DMA wait puts the clock gate at 2.4GHz when the stream starts.
            warm = xt_pool.tile([P, P], bf16, tag="warm")
            nc.vector.memset(warm, 0.0)
            wps = [psum_pool.tile([P, HSLAB], f32, tag="ps", name=f"warm_ps{i}")
                   for i in range(2)]
            for i in range(32):
                nc.tensor.matmul(wps[i % 2][:, :P], warm, warm,
                                 start=True, stop=True)

            for d in range(KT):
                nc.sync.dma_start(xta_sb[:, d, :], xta[d * P:(d + 1) * P, :])
                nc.sync.dma_start(wv0_sb[:, d, :], WV0[d * P:(d + 1) * P, :])
            for d in range(KT):
                nc.sync.dma_start(xtb_sb[:, d, :], xtb[d * P:(d + 1) * P, :])
            w00 = wv0_sb[:, :, :HSLAB]
            v00 = wv0_sb[:, :, HSLAB:]

            # jobs: (expert-slot, sub-slab, xt tile, tiles, c-tile base)
            jobs = [
                (0, 0, xta_sb, T1, 0),
                (0, 1, xta_sb, T1, 0),
                (1, 0, xtb_sb, T2, T1),
                (1, 1, xtb_sb, T2, T1),
            ]
            for slot, hs, xt_sb, tiles, cbase in jobs:
                if slot == 0 and hs == 0:
                    wsl, vsl = w00, v00
                else:
                    # Steady-state slabs: one big DMA each (prefetched
                    # ahead via the pool rotation).
                    wsl = wv_pool.tile([P, KT, HSLAB], bf16, tag="wv")
                    nc.sync.dma_start(
                        wsl, Wd[slot, hs].rearrange("(ko p) n -> p ko n", p=P)
                    )
                    vsl = wv_pool.tile([P, KT, HSLAB], bf16, tag="wv")
                    nc.sync.dma_start(
                        vsl, Vd[slot, hs].rearrange("(ko p) n -> p ko n", p=P)
                    )

                def a_phase(ct):
                    pa = psum_pool.tile([P, HSLAB], f32, tag="ps")
                    for d in range(KT):
                        nc.tensor.matmul(
                            pa, xt_sb[:, d, ct * P:(ct + 1) * P], wsl[:, d, :],
                            start=(d == 0), stop=(d == KT - 1),
                        )
                    return pa

                def b_phase_and_out(ct, pa, split=False):
                    c = cbase + ct
                    # For the very last tile, run the gate matmuls in four
                    # quarter-width accumulation groups so each SiLU/mul/DMA
                    # overlaps the next quarter's matmuls and the final DMA
                    # is small (shortens the kernel tail).
                    q = HSLAB // 4
                    halves = (
                        [(i * q, (i + 1) * q) for i in range(4)] if split
                        else [(0, HSLAB)]
                    )
                    for h0, h1 in halves:
                        nh = h1 - h0
                        pb = psum_pool.tile([P, HSLAB], f32, tag="ps")
                        for d in range(KT):
                            nc.tensor.matmul(
                                pb[:, :nh],
                                xt_sb[:, d, ct * P:(ct + 1) * P],
                                vsl[:, d, h0:h1],
                                start=(d == 0), stop=(d == KT - 1),
                            )
                        sil = elt_pool.tile([P, HSLAB], f32, tag="sil")
                        nc.scalar.activation(
                            sil[:, :nh], pb[:, :nh],
                            mybir.ActivationFunctionType.Silu,
                        )
                        ot = elt_pool.tile([P, HSLAB], f32, tag="ot")
                        nc.vector.tensor_mul(ot[:, :nh], pa[:, h0:h1], sil[:, :nh])
                        nc.sync.dma_start(
                            out[c * P:(c + 1) * P,
                                hs * HSLAB + h0:hs * HSLAB + h1],
                            ot[:, :nh],
                        )

                # In the first job the V slab races the PE up the ramp:
                # lag its phases a few tiles behind the W phases to give
                # the v00 DMA extra arrival slack.
                lag = 3 if (slot == 0 and hs == 0) else 0
                last_ct = tiles - 1 if (slot == 1 and hs == 1) else -1
                pending = []
                for ct in range(tiles):
                    pending.append((ct, a_phase(ct)))
                    if len(pending) > lag:
                        pct, ppa = pending.pop(0)
                        b_phase_and_out(pct, ppa, split=(pct == last_ct))
                for pct, ppa in pending:
                    b_phase_and_out(pct, ppa, split=(pct == last_ct))
    nc.compile()
    return nc


def _get_kernel(T1, T2):
    key = (T1, T2)
    if key not in _compiled:
        _compiled[key] = _build(T1, T2)
    return _compiled[key]


def _route(xf, router_w, router_b, k):
    """fp32 router: per-expert token ids and softmax combine weights."""
    logits = xf @ router_w.astype(np.float32) + router_b.astype(np.float32)
    # stable: ties resolve to the lower expert index, like lax.top_k
    order = np.argsort(-logits, axis=1, kind="stable")[:, :k]   # [N, k]
    top_logits = np.take_along_axis(logits, order, axis=1)
    m = top_logits.max(axis=1, keepdims=True)
    p = np.exp(top_logits - m)
    p /= p.sum(axis=1, keepdims=True)                   # [N, k]
    ids, wts = [], []
    for e in range(E):
        mask = order == e                               # [N, k]
        tok = np.nonzero(mask.any(axis=1))[0]
        wt = (p * mask).sum(axis=1)[tok]
        ids.append(tok)
        wts.append(wt.astype(np.float32))
    return ids, wts


def run(inputs, trace=False, trace_cores=None):
    """Full pipeline. Returns (output, BassKernelResults)."""
    from concourse.bass_utils import run_bass_kernel_spmd

    x = np.asarray(inputs["x"], dtype=np.float32)
    W = np.asarray(inputs["W"], dtype=np.float32)
    V = np.asarray(inputs["V"], dtype=np.float32)
    router_w = np.asarray(inputs["router_w"])
    router_b = np.asarray(inputs["router_b"])
    k = int(np.asarray(inputs["top_k"]))

    B, T, d = x.shape
    assert d == D and W.shape == (E, D, H) and V.shape == (E, D, H)
    N = B * T
    xf = x.reshape(N, D)

    ids, wts = _route(xf, router_w, router_b, k)
    tcounts = [max(1, -(-len(i) // P)) for i in ids]     # tiles per expert

    # Pair the largest-tile expert with the smallest, 2nd with 2nd-smallest...
    order = sorted(range(E), key=lambda e: -tcounts[e])
    pairs = [(order[i], order[E - 1 - i]) for i in range(E // 2)]
    T1 = max(tcounts[a] for a, _ in pairs)
    T2 = max(tcounts[b] for _, b in pairs)
    C1, C2 = T1 * P, T2 * P

    nc = _get_kernel(T1, T2)

    bf16 = ml_dtypes.bfloat16
    # Per-expert weight slabs [4, D, 512] in bf16, cast once.
    Wr = [np.ascontiguousarray(
        W[e].reshape(D, 4, HSLAB).transpose(1, 0, 2)).astype(bf16)
        for e in range(E)]
    Vr = [np.ascontiguousarray(
        V[e].reshape(D, 4, HSLAB).transpose(1, 0, 2)).astype(bf16)
        for e in range(E)]

    in_maps = []
    for a, b in pairs:
        na, nb = len(ids[a]), len(ids[b])
        xta = np.zeros((D, C1), dtype=bf16)
        xta[:, :na] = xf[ids[a]].T.astype(bf16)
        xtb = np.zeros((D, C2), dtype=bf16)
        xtb[:, :nb] = xf[ids[b]].T.astype(bf16)
        for h in range(2):
            w_core = np.stack([Wr[a][2 * h:2 * h + 2], Wr[b][2 * h:2 * h + 2]])
            v_core = np.stack([Vr[a][2 * h:2 * h + 2], Vr[b][2 * h:2 * h + 2]])
            wv0 = np.concatenate([Wr[a][2 * h], Vr[a][2 * h]], axis=1)
            in_maps.append({"xta": xta, "xtb": xtb, "w": w_core, "v": v_core,
                            "wv0": np.ascontiguousarray(wv0)})

    res = run_bass_kernel_spmd(
        nc,
        in_maps,
        core_ids=list(range(N_CORES)),
        trace=trace,
        trace_cores=trace_cores,
    )

    outf = np.zeros((N, H), dtype=np.float32)
    for p, (a, b) in enumerate(pairs):
        lo = res.results[2 * p]["out"]                  # [C, 1024] h-half 0
        hi = res.results[2 * p + 1]["out"]              # [C, 1024] h-half 1
        na, nb = len(ids[a]), len(ids[b])
        if na:
            y = np.concatenate([lo[:na], hi[:na]], axis=1)
            outf[ids[a]] += y * wts[a][:, None]
        if nb:
            y = np.concatenate([lo[C1:C1 + nb], hi[C1:C1 + nb]], axis=1)
            outf[ids[b]] += y * wts[b][:, None]
    return outf.reshape(B, T, H), res


def kernel(**inputs):
    out, _ = run(inputs, trace=False)
    return out


# revision 28
# speedup vs baseline: 1.0115x; 1.0115x over previous
"""Trainium2 Bass kernel for nn_MixtureOfExperts (moe_routing).

Strategy (expert-parallel + H-split, derived from the sharding hint):
  - Host computes the tiny router (N x D @ D x E = 0.1% of total FLOPs),
    top-k selection and softmax combine weights in fp32 numpy.
  - Experts are paired so their token-tile counts balance (a 9-tile
    expert with an 8-tile one).  Each pair is served by TWO cores, each
    computing one H-half (1024 cols) of BOTH experts of the pair:
        out_e = (x_e @ W_e) * silu(x_e @ V_e)
    This equalizes per-core matmul counts across all 8 cores at
    (T1+T2) token-tiles x (H/2) columns, below the max-expert padding
    cost of plain expert-parallel.  Matmuls run in bf16 (fp32 PSUM).
  - Host scatter-adds the per-expert outputs weighted by the combine
    probabilities (the reference's zeros+scatter-add semantics).

Device kernel layout per core (pair (a,b), h-half h):
  xta  [D, T1*128] bf16   tokens routed to expert a (transposed)
  xtb  [D, T2*128] bf16   tokens routed to expert b
  w,v  [2, 2, D, 512] bf16  h-slab-major weights: [slot, sub-slab, D, 512]
  out  [(T1+T2)*128, 1024] fp32
  Loop: per (expert-slot, 512-col sub-slab), per 128-token tile:
  16 accumulating matmuls per matrix (lhsT = stationary x^T tile,
  rhs = weight slab slice), then ACT silu + DVE multiply, DMA out.
"""

import numpy as np
import ml_dtypes

P = 128
D = 2048
H = 2048
E = 8
N_CORES = 8
HSLAB = 512
HHALF = 1024

_compiled = {}


def _build(T1, T2):
    """Build the Bass/Tile kernel for token-tile counts (T1, T2)."""
    import concourse.bacc as bacc
    import concourse.mybir as mybir
    import concourse.tile as tile

    KT = D // P              # 16 contraction tiles
    C1, C2 = T1 * P, T2 * P
    C = C1 + C2

    nc = bacc.Bacc("TRN2", target_bir_lowering=False, debug=False)
    bf16 = mybir.dt.bfloat16
    f32 = mybir.dt.float32

    # SBUF budget guard (KB/partition): resident x^T costs 4*(T1+T2),
    # each weight-slab buffer 16, elementwise pools ~24.  Shrink slab
    # prefetch depth if a skewed routing inflates the token capacity.
    wv_bufs = 5
    while 4 * (T1 + T2) + 16 * wv_bufs + 28 > 188 and wv_bufs > 2:
        wv_bufs -= 1

    xta = nc.dram_tensor("xta", [D, C1], bf16, kind="ExternalInput").ap()
    xtb = nc.dram_tensor("xtb", [D, C2], bf16, kind="ExternalInput").ap()
    Wd = nc.dram_tensor("w", [2, 2, D, HSLAB], bf16, kind="ExternalInput").ap()
    Vd = nc.dram_tensor("v", [2, 2, D, HSLAB], bf16, kind="ExternalInput").ap()
    # First W/V slab pair pre-fused host-side: doubles bytes-per-dma_start
    # during the ramp chase, which is issue-rate-bound (~0.6us/issue).
    WV0 = nc.dram_tensor("wv0", [D, 2 * HSLAB], bf16, kind="ExternalInput").ap()
    out = nc.dram_tensor("out", [C, HHALF], f32, kind="ExternalOutput").ap()

    with tile.TileContext(nc) as tc:
        with (
            tc.tile_pool(name="xt", bufs=1) as xt_pool,
            tc.tile_pool(name="wv", bufs=wv_bufs) as wv_pool,
            tc.tile_pool(name="elt", bufs=5) as elt_pool,
            tc.tile_pool(name="psum", bufs=8, space="PSUM") as psum_pool,
        ):
            # Resident x^T for both experts of the pair.  Interleave the
            # ramp-critical DMAs (xta, first W slab, first V slab) in
            # d-order so the PE starts the first accumulation chain as
            # data arrives; xtb streams later (needed after T1 tiles).
            xta_sb = xt_pool.tile([P, KT, C1], bf16, tag="xta")
            xtb_sb = xt_pool.tile([P, KT, C2], bf16, tag="xtb")
            wv0_sb = wv_pool.tile([P, KT, 2 * HSLAB], bf16, tag="wv0", bufs=1)

            # HAM pre-warm v2: back-to-back N=128 matmuls ALTERNATING psum
            # banks (a single bank serializes on WAW+drain and never trips
            # the activity window).  ~3.4us of dense PE activity during the
            # DMA wait puts the clock gate at 2.4GHz when the stream starts.
            warm = xt_pool.tile([P, P], bf16, tag="warm")
            nc.vector.memset(warm, 0.0)
            wps = [psum_pool.tile([P, HSLAB], f32, tag="ps", name=f"warm_ps{i}")
                   for i in range(2)]
            for i in range(32):
                nc.tensor.matmul(wps[i % 2][:, :P], warm, warm,
                                 start=True, stop=True)

            for d in range(KT):
                nc.sync.dma_start(xta_sb[:, d, :], xta[d * P:(d + 1) * P, :])
                nc.sync.dma_start(wv0_sb[:, d, :], WV0[d * P:(d + 1) * P, :])
            w00 = wv0_sb[:, :, :HSLAB]
            v00 = wv0_sb[:, :, HSLAB:]

            # jobs: (expert-slot, sub-slab, xt tile, tiles, c-tile base)
            jobs = [
                (0, 0, xta_sb, T1, 0),
                (0, 1, xta_sb, T1, 0),
                (1, 0, xtb_sb, T2, T1),
                (1, 1, xtb_sb, T2, T1),
            ]
            for slot, hs, xt_sb, tiles, cbase in jobs:
                if slot == 0 and hs == 0:
                    wsl, vsl = w00, v00
                else:
                    if slot == 0 and hs == 1:
                        # xtb deferred past the startup chase: both cores of
                        # an HBM-stack pair chase simultaneously (same SPMD
                        # program), and the extra 4.6MB pushed pair demand
                        # past stack supply (~5us straggler penalty).  It is
                        # not needed until job (1,0), far later.
                        for d in range(KT):
                            nc.sync.dma_start(
                                xtb_sb[:, d, :], xtb[d * P:(d + 1) * P, :]
                            )
                    # Steady-state slabs: one big DMA each (prefetched
                    # ahead via the pool rotation).
                    wsl = wv_pool.tile([P, KT, HSLAB], bf16, tag="wv")
                    nc.sync.dma_start(
                        wsl, Wd[slot, hs].rearrange("(ko p) n -> p ko n", p=P)
                    )
                    vsl = wv_pool.tile([P, KT, HSLAB], bf16, tag="wv")
                    nc.sync.dma_start(
                        vsl, Vd[slot, hs].rearrange("(ko p) n -> p ko n", p=P)
                    )

                def a_phase(ct):
                    pa = psum_pool.tile([P, HSLAB], f32, tag="ps")
                    for d in range(KT):
                        nc.tensor.matmul(
                            pa, xt_sb[:, d, ct * P:(ct + 1) * P], wsl[:, d, :],
                            start=(d == 0), stop=(d == KT - 1),
                        )
                    return pa

                def b_phase_and_out(ct, pa, split=False):
                    c = cbase + ct
                    # For the very last tile, run the gate matmuls in four
                    # quarter-width accumulation groups so each SiLU/mul/DMA
                    # overlaps the next quarter's matmuls and the final DMA
                    # is small (shortens the kernel tail).
                    q = HSLAB // 4
                    halves = (
                        [(i * q, (i + 1) * q) for i in range(4)] if split
                        else [(0, HSLAB)]
                    )
                    for h0, h1 in halves:
                        nh = h1 - h0
                        pb = psum_pool.tile([P, HSLAB], f32, tag="ps")
                        for d in range(KT):
                            nc.tensor.matmul(
                                pb[:, :nh],
                                xt_sb[:, d, ct * P:(ct + 1) * P],
                                vsl[:, d, h0:h1],
                                start=(d == 0), stop=(d == KT - 1),
                            )
                        sil = elt_pool.tile([P, HSLAB], f32, tag="sil")
                        nc.scalar.activation(
                            sil[:, :nh], pb[:, :nh],
                            mybir.ActivationFunctionType.Silu,
                        )
                        ot = elt_pool.tile([P, HSLAB], f32, tag="ot")
                        nc.vector.tensor_mul(ot[:, :nh], pa[:, h0:h1], sil[:, :nh])
                        nc.sync.dma_start(
                            out[c * P:(c + 1) * P,
                                hs * HSLAB + h0:hs * HSLAB + h1],
                            ot[:, :nh],
                        )

                # In the first job the V slab races the PE up the ramp:
                # lag its phases a few tiles behind the W phases to give
                # the v00 DMA extra arrival slack.
                lag = 3 if (slot == 0 and hs == 0) else 0
                last_ct = tiles - 1 if (slot == 1 and hs == 1) else -1
                pending = []
                for ct in range(tiles):
                    pending.append((ct, a_phase(ct)))
                    if len(pending) > lag:
                        pct, ppa = pending.pop(0)
                        b_phase_and_out(pct, ppa, split=(pct == last_ct))
                for pct, ppa in pending:
                    b_phase_and_out(pct, ppa, split=(pct == last_ct))
    nc.compile()
    return nc


def _get_kernel(T1, T2):
    key = (T1, T2)
    if key not in _compiled:
        _compiled[key] = _build(T1, T2)
    return _compiled[key]


def _route(xf, router_w, router_b, k):
    """fp32 router: per-expert token ids and softmax combine weights."""
    logits = xf @ router_w.astype(np.float32) + router_b.astype(np.float32)
    # stable: ties resolve to the lower expert index, like lax.top_k
    order = np.argsort(-logits, axis=1, kind="stable")[:, :k]   # [N, k]
    top_logits = np.take_along_axis(logits, order, axis=1)
    m = top_logits.max(axis=1, keepdims=True)
    p = np.exp(top_logits - m)
    p /= p.sum(axis=1, keepdims=True)                   # [N, k]
    ids, wts = [], []
    for e in range(E):
        mask = order == e                               # [N, k]
        tok = np.nonzero(mask.any(axis=1))[0]
        wt = (p * mask).sum(axis=1)[tok]
        ids.append(tok)
        wts.append(wt.astype(np.float32))
    return ids, wts


def run(inputs, trace=False, trace_cores=None):
    """Full pipeline. Returns (output, BassKernelResults)."""
    from concourse.bass_utils import run_bass_kernel_spmd

    x = np.asarray(inputs["x"], dtype=np.float32)
    W = np.asarray(inputs["W"], dtype=np.float32)
    V = np.asarray(inputs["V"], dtype=np.float32)
    router_w = np.asarray(inputs["router_w"])
    router_b = np.asarray(inputs["router_b"])
    k = int(np.asarray(inputs["top_k"]))

    B, T, d = x.shape
    assert d == D and W.shape == (E, D, H) and V.shape == (E, D, H)
    N = B * T
    xf = x.reshape(N, D)

    ids, wts = _route(xf, router_w, router_b, k)
    tcounts = [max(1, -(-len(i) // P)) for i in ids]     # tiles per expert

    # Pair the largest-tile expert with the smallest, 2nd with 2nd-smallest...
    order = sorted(range(E), key=lambda e: -tcounts[e])
    pairs = [(order[i], order[E - 1 - i]) for i in range(E // 2)]
    T1 = max(tcounts[a] for a, _ in pairs)
    T2 = max(tcounts[b] for _, b in pairs)
    C1, C2 = T1 * P, T2 * P

    nc = _get_kernel(T1, T2)

    bf16 = ml_dtypes.bfloat16
    # Per-expert weight slabs [4, D, 512] in bf16, cast once.
    Wr = [np.ascontiguousarray(
        W[e].reshape(D, 4, HSLAB).transpose(1, 0, 2)).astype(bf16)
        for e in range(E)]
    Vr = [np.ascontiguousarray(
        V[e].reshape(D, 4, HSLAB).transpose(1, 0, 2)).astype(bf16)
        for e in range(E)]

    in_maps = []
    for a, b in pairs:
        na, nb = len(ids[a]), len(ids[b])
        xta = np.zeros((D, C1), dtype=bf16)
        xta[:, :na] = xf[ids[a]].T.astype(bf16)
        xtb = np.zeros((D, C2), dtype=bf16)
        xtb[:, :nb] = xf[ids[b]].T.astype(bf16)
        for h in range(2):
            w_core = np.stack([Wr[a][2 * h:2 * h + 2], Wr[b][2 * h:2 * h + 2]])
            v_core = np.stack([Vr[a][2 * h:2 * h + 2], Vr[b][2 * h:2 * h + 2]])
            wv0 = np.concatenate([Wr[a][2 * h], Vr[a][2 * h]], axis=1)
            in_maps.append({"xta": xta, "xtb": xtb, "w": w_core, "v": v_core,
                            "wv0": np.ascontiguousarray(wv0)})

    res = run_bass_kernel_spmd(
        nc,
        in_maps,
        core_ids=list(range(N_CORES)),
        trace=trace,
        trace_cores=trace_cores,
    )

    outf = np.zeros((N, H), dtype=np.float32)
    for p, (a, b) in enumerate(pairs):
        lo = res.results[2 * p]["out"]                  # [C, 1024] h-half 0
        hi = res.results[2 * p + 1]["out"]              # [C, 1024] h-half 1
        na, nb = len(ids[a]), len(ids[b])
        if na:
            y = np.concatenate([lo[:na], hi[:na]], axis=1)
            outf[ids[a]] += y * wts[a][:, None]
        if nb:
            y = np.concatenate([lo[C1:C1 + nb], hi[C1:C1 + nb]], axis=1)
            outf[ids[b]] += y * wts[b][:, None]
    return outf.reshape(B, T, H), res


def kernel(**inputs):
    out, _ = run(inputs, trace=False)
    return out


# revision 32
# speedup vs baseline: 1.0192x; 1.0075x over previous
"""Trainium2 Bass kernel for nn_MixtureOfExperts (moe_routing).

Strategy (expert-parallel + H-split, derived from the sharding hint):
  - Host computes the tiny router (N x D @ D x E = 0.1% of total FLOPs),
    top-k selection and softmax combine weights in fp32 numpy.
  - Experts are paired so their token-tile counts balance (a 9-tile
    expert with an 8-tile one).  Each pair is served by TWO cores, each
    computing one H-half (1024 cols) of BOTH experts of the pair:
        out_e = (x_e @ W_e) * silu(x_e @ V_e)
    This equalizes per-core matmul counts across all 8 cores at
    (T1+T2) token-tiles x (H/2) columns, below the max-expert padding
    cost of plain expert-parallel.  Matmuls run in bf16 (fp32 PSUM).
  - Host scatter-adds the per-expert outputs weighted by the combine
    probabilities (the reference's zeros+scatter-add semantics).

Device kernel layout per core (pair (a,b), h-half h):
  xta  [D, T1*128] bf16   tokens routed to expert a (transposed)
  xtb  [D, T2*128] bf16   tokens routed to expert b
  w,v  [2, 2, D, 512] bf16  h-slab-major weights: [slot, sub-slab, D, 512]
  out  [(T1+T2)*128, 1024] fp32
  Loop: per (expert-slot, 512-col sub-slab), per 128-token tile:
  16 accumulating matmuls per matrix (lhsT = stationary x^T tile,
  rhs = weight slab slice), then ACT silu + DVE multiply, DMA out.
"""

import numpy as np
import ml_dtypes

P = 128
D = 2048
H = 2048
E = 8
N_CORES = 8
HSLAB = 512
HHALF = 1024

_compiled = {}


def _build(T1, T2):
    """Build the Bass/Tile kernel for token-tile counts (T1, T2)."""
    import concourse.bacc as bacc
    import concourse.mybir as mybir
    import concourse.tile as tile

    KT = D // P              # 16 contraction tiles
    C1, C2 = T1 * P, T2 * P
    C = C1 + C2

    nc = bacc.Bacc("TRN2", target_bir_lowering=False, debug=False)
    bf16 = mybir.dt.bfloat16
    f32 = mybir.dt.float32

    # SBUF budget guard (KB/partition): resident x^T costs 4*(T1+T2),
    # each weight-slab buffer 16, elementwise pools ~24.  Shrink slab
    # prefetch depth if a skewed routing inflates the token capacity.
    wv_bufs = 5
    while 4 * (T1 + T2) + 16 * wv_bufs + 28 > 188 and wv_bufs > 2:
        wv_bufs -= 1

    xtb = nc.dram_tensor("xtb", [D, C2], bf16, kind="ExternalInput").ap()
    Wd = nc.dram_tensor("w", [2, 2, D, HSLAB], bf16, kind="ExternalInput").ap()
    Vd = nc.dram_tensor("v", [2, 2, D, HSLAB], bf16, kind="ExternalInput").ap()
    # Whole startup chase host-fused into ONE tensor (xta | W00 | V00 per
    # d-row): the chase is dma_start-issue-rate bound (~0.6us/issue), so
    # 16 issues of 0.54MB beat 32 of ~0.27MB and shorten the window where
    # paired cores contend for their shared HBM stack.
    XW0 = nc.dram_tensor("xw0", [D, C1 + 2 * HSLAB], bf16,
                         kind="ExternalInput").ap()
    out = nc.dram_tensor("out", [C, HHALF], f32, kind="ExternalOutput").ap()

    with tile.TileContext(nc) as tc:
        with (
            tc.tile_pool(name="xt", bufs=1) as xt_pool,
            tc.tile_pool(name="wv", bufs=wv_bufs) as wv_pool,
            tc.tile_pool(name="elt", bufs=5) as elt_pool,
            tc.tile_pool(name="psum", bufs=8, space="PSUM") as psum_pool,
        ):
            # Resident x^T for both experts of the pair.  Interleave the
            # ramp-critical DMAs (xta, first W slab, first V slab) in
            # d-order so the PE starts the first accumulation chain as
            # data arrives; xtb streams later (needed after T1 tiles).
            xw0_sb = xt_pool.tile([P, KT, C1 + 2 * HSLAB], bf16, tag="xw0")
            xtb_sb = xt_pool.tile([P, KT, C2], bf16, tag="xtb")
            xta_sb = xw0_sb[:, :, :C1]

            # HAM pre-warm v2: back-to-back N=128 matmuls ALTERNATING psum
            # banks (a single bank serializes on WAW+drain and never trips
            # the activity window).  ~3.4us of dense PE activity during the
            # DMA wait puts the clock gate at 2.4GHz when the stream starts.
            warm = xt_pool.tile([P, P], bf16, tag="warm")
            nc.vector.memset(warm, 0.0)
            wps = [psum_pool.tile([P, HSLAB], f32, tag="ps", name=f"warm_ps{i}")
                   for i in range(2)]
            for i in range(32):
                nc.tensor.matmul(wps[i % 2][:, :P], warm, warm,
                                 start=True, stop=True)

            for d in range(KT):
                nc.sync.dma_start(xw0_sb[:, d, :], XW0[d * P:(d + 1) * P, :])
            w00 = xw0_sb[:, :, C1:C1 + HSLAB]
            v00 = xw0_sb[:, :, C1 + HSLAB:]

            # jobs: (expert-slot, sub-slab, xt tile, tiles, c-tile base)
            jobs = [
                (0, 0, xta_sb, T1, 0),
                (0, 1, xta_sb, T1, 0),
                (1, 0, xtb_sb, T2, T1),
                (1, 1, xtb_sb, T2, T1),
            ]
            for slot, hs, xt_sb, tiles, cbase in jobs:
                if slot == 0 and hs == 0:
                    wsl, vsl = w00, v00
                else:
                    if slot == 0 and hs == 1:
                        # xtb deferred past the startup chase: both cores of
                        # an HBM-stack pair chase simultaneously (same SPMD
                        # program), and the extra 4.6MB pushed pair demand
                        # past stack supply (~5us straggler penalty).  It is
                        # not needed until job (1,0), far later.
                        for d in range(KT):
                            nc.sync.dma_start(
                                xtb_sb[:, d, :], xtb[d * P:(d + 1) * P, :]
                            )
                    # Steady-state slabs: one big DMA each (prefetched
                    # ahead via the pool rotation).
                    wsl = wv_pool.tile([P, KT, HSLAB], bf16, tag="wv")
                    nc.sync.dma_start(
                        wsl, Wd[slot, hs].rearrange("(ko p) n -> p ko n", p=P)
                    )
                    vsl = wv_pool.tile([P, KT, HSLAB], bf16, tag="wv")
                    nc.sync.dma_start(
                        vsl, Vd[slot, hs].rearrange("(ko p) n -> p ko n", p=P)
                    )

                def a_phase(ct):
                    pa = psum_pool.tile([P, HSLAB], f32, tag="ps")
                    for d in range(KT):
                        nc.tensor.matmul(
                            pa, xt_sb[:, d, ct * P:(ct + 1) * P], wsl[:, d, :],
                            start=(d == 0), stop=(d == KT - 1),
                        )
                    return pa

                def b_phase_and_out(ct, pa, split=False):
                    c = cbase + ct
                    # For the very last tile, run the gate matmuls in four
                    # quarter-width accumulation groups so each SiLU/mul/DMA
                    # overlaps the next quarter's matmuls and the final DMA
                    # is small (shortens the kernel tail).
                    q = HSLAB // 4
                    halves = (
                        [(i * q, (i + 1) * q) for i in range(4)] if split
                        else [(0, HSLAB)]
                    )
                    for h0, h1 in halves:
                        nh = h1 - h0
                        pb = psum_pool.tile([P, HSLAB], f32, tag="ps")
                        for d in range(KT):
                            nc.tensor.matmul(
                                pb[:, :nh],
                                xt_sb[:, d, ct * P:(ct + 1) * P],
                                vsl[:, d, h0:h1],
                                start=(d == 0), stop=(d == KT - 1),
                            )
                        sil = elt_pool.tile([P, HSLAB], f32, tag="sil")
                        nc.scalar.activation(
                            sil[:, :nh], pb[:, :nh],
                            mybir.ActivationFunctionType.Silu,
                        )
                        ot = elt_pool.tile([P, HSLAB], f32, tag="ot")
                        nc.vector.tensor_mul(ot[:, :nh], pa[:, h0:h1], sil[:, :nh])
                        nc.sync.dma_start(
                            out[c * P:(c + 1) * P,
                                hs * HSLAB + h0:hs * HSLAB + h1],
                            ot[:, :nh],
                        )

                # In the first job the V slab races the PE up the ramp:
                # lag its phases a few tiles behind the W phases to give
                # the v00 DMA extra arrival slack.
                lag = 3 if (slot == 0 and hs == 0) else 0
                last_ct = tiles - 1 if (slot == 1 and hs == 1) else -1
                pending = []
                for ct in range(tiles):
                    pending.append((ct, a_phase(ct)))
                    if len(pending) > lag:
                        pct, ppa = pending.pop(0)
                        b_phase_and_out(pct, ppa, split=(pct == last_ct))
                for pct, ppa in pending:
                    b_phase_and_out(pct, ppa, split=(pct == last_ct))
    nc.compile()
    return nc


def _get_kernel(T1, T2):
    key = (T1, T2)
    if key not in _compiled:
        _compiled[key] = _build(T1, T2)
    return _compiled[key]


def _route(xf, router_w, router_b, k):
    """fp32 router: per-expert token ids and softmax combine weights."""
    logits = xf @ router_w.astype(np.float32) + router_b.astype(np.float32)
    # stable: ties resolve to the lower expert index, like lax.top_k
    order = np.argsort(-logits, axis=1, kind="stable")[:, :k]   # [N, k]
    top_logits = np.take_along_axis(logits, order, axis=1)
    m = top_logits.max(axis=1, keepdims=True)
    p = np.exp(top_logits - m)
    p /= p.sum(axis=1, keepdims=True)                   # [N, k]
    ids, wts = [], []
    for e in range(E):
        mask = order == e                               # [N, k]
        tok = np.nonzero(mask.any(axis=1))[0]
        wt = (p * mask).sum(axis=1)[tok]
        ids.append(tok)
        wts.append(wt.astype(np.float32))
    return ids, wts


def run(inputs, trace=False, trace_cores=None):
    """Full pipeline. Returns (output, BassKernelResults)."""
    from concourse.bass_utils import run_bass_kernel_spmd

    x = np.asarray(inputs["x"], dtype=np.float32)
    W = np.asarray(inputs["W"], dtype=np.float32)
    V = np.asarray(inputs["V"], dtype=np.float32)
    router_w = np.asarray(inputs["router_w"])
    router_b = np.asarray(inputs["router_b"])
    k = int(np.asarray(inputs["top_k"]))

    B, T, d = x.shape
    assert d == D and W.shape == (E, D, H) and V.shape == (E, D, H)
    N = B * T
    xf = x.reshape(N, D)

    ids, wts = _route(xf, router_w, router_b, k)
    tcounts = [max(1, -(-len(i) // P)) for i in ids]     # tiles per expert

    # Pair the largest-tile expert with the smallest, 2nd with 2nd-smallest...
    order = sorted(range(E), key=lambda e: -tcounts[e])
    pairs = [(order[i], order[E - 1 - i]) for i in range(E // 2)]
    T1 = max(tcounts[a] for a, _ in pairs)
    T2 = max(tcounts[b] for _, b in pairs)
    C1, C2 = T1 * P, T2 * P

    nc = _get_kernel(T1, T2)

    bf16 = ml_dtypes.bfloat16
    # Per-expert weight slabs [4, D, 512] in bf16, cast once.
    Wr = [np.ascontiguousarray(
        W[e].reshape(D, 4, HSLAB).transpose(1, 0, 2)).astype(bf16)
        for e in range(E)]
    Vr = [np.ascontiguousarray(
        V[e].reshape(D, 4, HSLAB).transpose(1, 0, 2)).astype(bf16)
        for e in range(E)]

    in_maps = []
    for a, b in pairs:
        na, nb = len(ids[a]), len(ids[b])
        xta = np.zeros((D, C1), dtype=bf16)
        xta[:, :na] = xf[ids[a]].T.astype(bf16)
        xtb = np.zeros((D, C2), dtype=bf16)
        xtb[:, :nb] = xf[ids[b]].T.astype(bf16)
        for h in range(2):
            w_core = np.stack([Wr[a][2 * h:2 * h + 2], Wr[b][2 * h:2 * h + 2]])
            v_core = np.stack([Vr[a][2 * h:2 * h + 2], Vr[b][2 * h:2 * h + 2]])
            xw0 = np.ascontiguousarray(
                np.concatenate([xta, Wr[a][2 * h], Vr[a][2 * h]], axis=1))
            in_maps.append({"xw0": xw0, "xtb": xtb,
                            "w": w_core, "v": v_core})

    res = run_bass_kernel_spmd(
        nc,
        in_maps,
        core_ids=list(range(N_CORES)),
        trace=trace,
        trace_cores=trace_cores,
    )

    outf = np.zeros((N, H), dtype=np.float32)
    for p, (a, b) in enumerate(pairs):
        lo = res.results[2 * p]["out"]                  # [C, 1024] h-half 0
        hi = res.results[2 * p + 1]["out"]              # [C, 1024] h-half 1
        na, nb = len(ids[a]), len(ids[b])
        if na:
            y = np.concatenate([lo[:na], hi[:na]], axis=1)
            outf[ids[a]] += y * wts[a][:, None]
        if nb:
            y = np.concatenate([lo[C1:C1 + nb], hi[C1:C1 + nb]], axis=1)
            outf[ids[b]] += y * wts[b][:, None]
    return outf.reshape(B, T, H), res


def kernel(**inputs):
    out, _ = run(inputs, trace=False)
    return out
